# revision 11
# baseline (speedup 1.0000x reference)
"""Trainium2 Bass kernel for nn_MicroAdder (tiny dense transformer).

Decomposition (v3):
  Every per-element quantity in the reference network is affine in the basis
  [u_s, w_s, 1] with u = cos(tok_angle), w = sin(tok_angle) and
  position-dependent constant coefficients.  The HOST gathers u/w from a
  14-entry table and ships the basis directly as fp16 [69, B] -- no on-device
  trig, no int->float casts, and the basis is already transposed so mm1 needs
  no PE transpose.  One PE matmul per 128-row block (lhsT = basis slice,
  rhs = R [69, 272]) produces all 8 linear forms (att, rr, q0, q1, e0, e1,
  y0, y1), with the causal-softmax attention mixing folded into R.  A short
  fp16 elementwise chain (DVE 2x modes; Ln/Exp for the two rsqrt's -- the DVE
  reciprocal is ~8 cycles/elem, Ln+Exp on ACT is ~3x cheaper) produces the two
  logit forms L0, L1.  These are DMA-transposed (XBAR) and expanded to the
  (T,V) logits by a second PE matmul with a block-diagonal constant rhs, then
  stored as bf16 (host converts to fp32).

Sharding: pure data parallel over the batch dim across 8 NeuronCores.
"""

import math
import sys

import numpy as np

for _p in ("/opt/trn_rl_repo", "/root/.axon_site/_ro/trn_rl_repo"):
    if _p not in sys.path:
        sys.path.append(_p)

import concourse.bacc as bacc  # noqa: E402
import concourse.bass as bass  # noqa: E402
import concourse.tile as tile  # noqa: E402
from concourse import mybir  # noqa: E402
from concourse.bass_utils import run_bass_kernel_spmd  # noqa: E402
from concourse.masks import make_identity  # noqa: E402

# ---------------------------------------------------------------- problem dims
B, T, V = 65536, 34, 14
D, EPS, MAX_DIGITS = 5, 1e-5, 10
NCORES = 8
BC = B // NCORES            # rows per core = 8192
P = 128                     # partitions
NBLK = BC // P              # 64 blocks of 128 rows per core
SGB = 16                    # blocks per supergroup
NSG = NBLK // SGB           # 4 supergroups
NQ = SGB // 4               # quads per supergroup = 4
K1 = 2 * T + 1              # basis size = 69
NG = 8                      # mm1 form groups
N1 = NG * T                 # 272
N2 = T * V                  # 476
FW = SGB * T                # 544 chain columns per supergroup
NPRM = 9

F32 = mybir.dt.float32
F16 = mybir.dt.float16
BF16 = mybir.dt.bfloat16
I32 = mybir.dt.int32
AF = mybir.ActivationFunctionType
ALU = mybir.AluOpType

# mm1 output group order (columns g*T..(g+1)*T of R).  ATT/Y0/Y1 adjacent so
# one ACT copy drains all three; the att*rr product then has only one PSUM
# operand (HW allows at most one non-scalar PSUM input per vector op).
G_ATT, G_Y0, G_Y1, G_RR, G_Q0, G_Q1, G_E0, G_E1 = range(8)

# PRM slots
P_RAT, P_SQ0, P_C3, P_H00, P_H10, P_H01, P_H11, P_EPS, P_ZERO = range(9)


# ---------------------------------------------------------------- host tables
def host_tables(tok_A, tok_start, tok_stride, sp_amp, sp_phase, sp_slope, sp_offset,
                norm_w, q_w, q_phase, out_A, out_B, fc1_w, fc2_w, head_w):
    f = np.float64
    A = f(tok_A)
    t = np.arange(T, dtype=f)
    th = 2.0 * np.pi * t / MAX_DIGITS + f(sp_phase)
    pos = np.stack([f(sp_amp) * np.cos(th), f(sp_amp) * np.sin(th),
                    f(sp_slope) * t + f(sp_offset)], axis=-1)
    k = pos @ np.asarray(q_w, f).T
    c0, s0 = np.cos(f(q_phase[0])), np.sin(f(q_phase[0]))
    q = k.copy()
    q[:, 0] = c0 * k[:, 0] - s0 * k[:, 1]
    q[:, 1] = s0 * k[:, 0] + c0 * k[:, 1]
    scores = (q @ k.T) / np.sqrt(f(5.0))
    sm = np.where(np.tril(np.ones((T, T), bool)), scores, -np.inf)
    sm = sm - sm.max(-1, keepdims=True)
    e = np.exp(sm)
    attn = e / e.sum(-1, keepdims=True)

    nw = np.asarray(norm_w, f)
    oA = np.asarray(out_A, f)[:, 0]
    oB = np.asarray(out_B, f)[0]
    S_t = A * A + (pos ** 2).sum(-1)          # |x|^2 per position (tok part = A^2)
    rms1 = np.sqrt(S_t / D + EPS)

    M0 = attn * (A * nw[0] * oA[0] / rms1)[None, :]
    M1 = attn * (A * nw[1] * oA[1] / rms1)[None, :]
    c_t = attn @ ((pos * (nw[2:] * oA[2:])[None, :]).sum(-1) / rms1)

    g0 = np.asarray(fc2_w, f)[:, 0]
    g1 = np.asarray(fc2_w, f)[:, 1]
    projs = {
        G_Q0: nw * np.asarray(fc1_w, f)[0],
        G_Q1: nw * np.asarray(fc1_w, f)[1],
        G_E0: 2.0 * g0,
        G_E1: 2.0 * g1,
        G_Y0: nw * np.asarray(head_w, f)[0],
        G_Y1: nw * np.asarray(head_w, f)[1],
    }
    # R columns: group g covers cols g*T..(g+1)*T; rows: u_s (0:T), w_s (T:2T),
    # const (2T).  att-form = z (attention scalar), rr-form = 2 x.oB + |oB|^2 z.
    R = np.zeros((K1, NG * T), dtype=f)
    dd = np.eye(T, dtype=f)
    b2 = (oB ** 2).sum()
    for gi in range(NG):
        cols = slice(gi * T, (gi + 1) * T)
        if gi == G_ATT:
            R[0:T, cols] = M0.T
            R[T:2 * T, cols] = M1.T
            R[2 * T, cols] = c_t
        elif gi == G_RR:
            R[0:T, cols] = 2 * A * oB[0] * dd + b2 * M0.T
            R[T:2 * T, cols] = 2 * A * oB[1] * dd + b2 * M1.T
            R[2 * T, cols] = 2 * (pos * oB[None, 2:]).sum(-1) + b2 * c_t
        else:
            v = projs[gi]
            bv = (oB * v).sum()
            R[0:T, cols] = A * v[0] * dd + bv * M0.T
            R[T:2 * T, cols] = A * v[1] * dd + bv * M1.T
            R[2 * T, cols] = (pos * v[None, 2:]).sum(-1) + bv * c_t

    G00, G01, G11 = (g0 * g0).sum(), (g0 * g1).sum(), (g1 * g1).sum()
    if G00 > 1e-30:
        sq0, rat = np.sqrt(G00), G01 / G00
        c3 = np.sqrt(max(G11 - G01 * G01 / G00, 0.0))
    else:
        sq0, rat, c3 = 0.0, 0.0, np.sqrt(G11)
    hv0 = nw * np.asarray(head_w, f)[0]
    hv1 = nw * np.asarray(head_w, f)[1]
    H = np.array([[(g0 * hv0).sum(), (g0 * hv1).sum()],
                  [(g1 * hv0).sum(), (g1 * hv1).sum()]])

    dvoc = np.arange(V, dtype=f)
    ang = f(tok_start) + dvoc * f(tok_stride)
    E = np.stack([A * np.cos(ang), A * np.sin(ang)], axis=-1)   # (V, 2)
    RHS2 = np.zeros((2 * T, N2), dtype=f)
    for t_ in range(T):
        RHS2[t_, t_ * V:(t_ + 1) * V] = E[:, 0]
        RHS2[T + t_, t_ * V:(t_ + 1) * V] = E[:, 1]

    # token basis tables: u = cos(ang_v), w = sin(ang_v)
    CU = np.cos(ang)
    SW = np.sin(ang)

    SROW = np.tile(S_t, SGB)[None, :]  # [1, 544]
    PRM = np.zeros((1, NPRM), dtype=f)
    PRM[0, P_RAT] = rat
    PRM[0, P_SQ0] = sq0
    PRM[0, P_C3] = c3
    PRM[0, P_H00] = H[0, 0]
    PRM[0, P_H10] = H[1, 0]
    PRM[0, P_H01] = H[0, 1]
    PRM[0, P_H11] = H[1, 1]
    PRM[0, P_EPS] = EPS
    PRM[0, P_ZERO] = 0.0
    return (R.astype(np.float16), RHS2.astype(np.float16),
            np.ascontiguousarray(SROW, np.float16).copy(),
            np.ascontiguousarray(PRM, np.float32).copy(),
            CU.astype(np.float16), SW.astype(np.float16))


def build_basis(idx, CU, SW):
    """[69, B] fp16: rows 0:T = u_t(b), rows T:2T = w_t(b), row 2T = 1."""
    bset = np.empty((K1, idx.shape[0]), np.float16)
    bset[0:T, :] = CU[idx].T
    bset[T:2 * T, :] = SW[idx].T
    bset[2 * T, :] = np.float16(1.0)
    return bset


def act_raw(nc, out, in_, func, bias, scale):
    """Emit InstActivation directly (same lowering as BassScalarEngine.
    activation) for funcs the wrapper refuses (Rsqrt)."""
    se = nc.scalar
    inputs = [se.lower_ap(in_)]
    for arg in (bias, scale, 0.0):
        if isinstance(arg, bass.AP):
            inputs.append(se.lower_ap(arg))
        else:
            inputs.append(mybir.ImmediateValue(dtype=mybir.dt.float32, value=arg))
    return se.add_instruction(mybir.InstActivation(
        name=se.bass.get_next_instruction_name(),
        func=func, ins=inputs, outs=[se.lower_ap(out)]))


# ---------------------------------------------------------------- bass kernel
def build_bass(prm_vals):
    nc = bacc.Bacc("TRN2", target_bir_lowering=False, debug=False)

    basis_d = nc.dram_tensor("basis", [K1, BC], F16, kind="ExternalInput").ap()
    r_d = nc.dram_tensor("R", [K1, N1], F16, kind="ExternalInput").ap()
    rhs2_d = nc.dram_tensor("RHS2", [2 * T, N2], F16, kind="ExternalInput").ap()
    srow_d = nc.dram_tensor("SROW", [1, FW], F16, kind="ExternalInput").ap()
    prm_d = nc.dram_tensor("PRM", [1, NPRM], F32, kind="ExternalInput").ap()
    out_d = nc.dram_tensor("out", [BC, N2], BF16, kind="ExternalOutput").ap()

    # DRAM out view: block n covers batch rows n*128..n*128+127, partition p
    # holds row n*128+p (matches mm1 lhsT = basis columns n*128+p)
    out_v = out_d.rearrange("(n p) c -> p n c", p=P)       # [128, 64, 476]

    with tile.TileContext(nc) as tc:
        with (
            tc.tile_pool(name="const", bufs=1) as cpool,
            tc.tile_pool(name="bas", bufs=2) as basp,
            tc.tile_pool(name="dr", bufs=2) as drp,
            tc.tile_pool(name="ch", bufs=2) as chp,
            tc.tile_pool(name="ltT", bufs=4) as ltp,
            tc.tile_pool(name="outsb", bufs=3) as outp,
            tc.tile_pool(name="pmm1", bufs=1, space="PSUM") as pmm1p,
            tc.tile_pool(name="pout", bufs=1, space="PSUM") as poutp,
            tc.tile_pool(name="ptr", bufs=2, space="PSUM") as ptrp,
        ):
            # ---- constants
            ident = cpool.tile([P, P], F16)
            make_identity(nc, ident[:])
            r_sb = cpool.tile([K1, N1], F16)
            nc.sync.dma_start(r_sb[:], r_d)
            rhs2_sb = cpool.tile([2 * T, N2], F16)
            nc.sync.dma_start(rhs2_sb[:], rhs2_d)
            s_sb = cpool.tile([P, FW], F16)
            nc.sync.dma_start(s_sb[:], srow_d.broadcast_to([P, FW]))
            prm_sb = cpool.tile([P, NPRM], F32)
            nc.sync.dma_start(prm_sb[:], prm_d.broadcast_to([P, NPRM]))

            def prm(i):
                return prm_sb[:, i:i + 1]

            def pv(i):
                return float(prm_vals[i])

            # persistent double-buffered L-form tiles; cols 68:128 of each
            # block stay zero (transposed padding rows are never read by mm2,
            # but the XBAR transpose reads them)
            lints = [cpool.tile([P, SGB * P], F16, tag=f"lint{i}", name=f"lint{i}")
                     for i in range(2)]
            for lt in lints:
                nc.vector.memset(lt[:], 0.0)

            # state carried between supergroups for the pipelined F phase
            prev = None  # (lint3 view, j0 of previous supergroup)

            def emit_F_quad(lint3, j0, q):
                """Transpose + mm2 + drain + store for blocks q*4..q*4+3."""
                pt = ptrp.tile([P, 4 * P], F16, tag="pt")
                pt4 = pt[:].rearrange("p (k c) -> p k c", k=4)
                for k in range(4):
                    nc.tensor.transpose(pt4[:, k, :], lint3[:, q * 4 + k, :],
                                        ident[:])
                lt = ltp.tile([P, 4 * P], F16, tag="ltT")
                nc.vector.tensor_copy(lt[:], pt[:])
                lt4 = lt[:].rearrange("p (k c) -> p k c", k=4)
                for h in range(2):
                    po = poutp.tile([P, 2 * 512], F32, tag="po")
                    po2 = po[:].rearrange("p (k c) -> p k c", k=2)
                    for k in range(2):
                        nc.tensor.matmul(po2[:, k, 0:N2],
                                         lt4[0:2 * T, 2 * h + k, :],
                                         rhs2_sb[:], start=True, stop=True)
                    o_sb = outp.tile([P, 2, N2], BF16, tag="osb")
                    eng = (nc.vector, nc.scalar, nc.scalar, nc.vector)[(2 * q + h) % 4]
                    if eng is nc.scalar:
                        nc.scalar.copy(o_sb[:], po2[:, :, 0:N2])
                    else:
                        eng.tensor_copy(o_sb[:], po2[:, :, 0:N2])
                    jb = j0 + q * 4 + 2 * h
                    nc.sync.dma_start(out_v[:, jb:jb + 2, :], o_sb[:])

            # prefetch all basis tiles up front
            b_ts = []
            for sg in range(NSG):
                bt = basp.tile([K1, SGB * P], F16, tag=f"bas{sg}", name=f"bas{sg}")
                nc.sync.dma_start(bt[:], basis_d[:, sg * SGB * P:(sg + 1) * SGB * P])
                b_ts.append(bt)

            for sg in range(NSG):
                j0 = sg * SGB
                lint = lints[sg % 2]
                lint3 = lint[:].rearrange("p (j c) -> p j c", c=P)
                b_t = b_ts[sg]

                # drain targets for the whole supergroup (fp16)
                ayy = drp.tile([P, SGB, 3 * T], F16, tag="ayy")
                ar = drp.tile([P, SGB, T], F16, tag="ar")
                rho = drp.tile([P, SGB, 2 * T], F16, tag="rho")
                tab = drp.tile([P, SGB, 2 * T], F16, tag="tab")

                # ---------------- phase B: mm1 per quad + drains
                for q in range(NQ):
                    pm = pmm1p.tile([P, 4 * 512], F32, tag="mm1")
                    pm4 = pm[:].rearrange("p (k c) -> p k c", k=4)
                    for k in range(4):
                        j = q * 4 + k
                        nc.tensor.matmul(
                            pm4[:, k, 0:N1],
                            b_t[:, j * P:(j + 1) * P],
                            r_sb[:], start=True, stop=True)
                    qs = slice(q * 4, q * 4 + 4)

                    def g(g0, g1=None):
                        g1 = g0 if g1 is None else g1
                        return pm4[:, :, g0 * T:(g1 + 1) * T]

                    nc.scalar.copy(ayy[:, qs, :], g(G_ATT, G_Y1))
                    nc.vector.tensor_mul(ar[:, qs, :], ayy[:, qs, 0:T], g(G_RR))
                    nc.vector.tensor_scalar_max(rho[:, qs, :], g(G_Q0, G_Q1), 0.0)
                    nc.vector.tensor_mul(tab[:, qs, :], rho[:, qs, :], g(G_E0, G_E1))

                # ---------------- phase F of the previous supergroup
                if prev is not None:
                    for q in range(NQ):
                        emit_F_quad(prev[0], prev[1], q)

                # ---------------- phase D: fp16 chain on [128, 544]
                # restructured for dependency depth: everything derivable from
                # rho/tab/ayy runs before (or concurrent with) the first Rsqrt;
                # the critical path is n2 -> inv2 -> isq -> qs2 -> n3 -> inv3
                # -> L0/L1.
                arf = ar[:].rearrange("p j t -> p (j t)")
                rho0 = rho[:, :, 0:T]
                rho1 = rho[:, :, T:2 * T]
                y0 = ayy[:, :, T:2 * T]
                y1 = ayy[:, :, 2 * T:3 * T]

                def ct(tag):
                    t_ = chp.tile([P, FW], F16, tag=tag)
                    return t_, t_[:].rearrange("p (j t) -> p j t", t=T)

                n2, n2v = ct("n2")
                nc.vector.tensor_add(n2[:], arf, s_sb[:])
                inv2, inv2v = ct("inv2")
                act_raw(nc, inv2[:], n2[:], AF.Rsqrt, prm(P_EPS), 1.0 / D)

                # pre-inv2 work (off critical path)
                pre_v, pre_vv = ct("pre_v")
                nc.vector.tensor_scalar(pre_v[:], rho1, pv(P_RAT), None,
                                        op0=ALU.mult)
                nc.vector.tensor_add(pre_vv, pre_vv, rho0)
                sqA, _ = ct("sqA")
                nc.scalar.activation(sqA[:], pre_v[:], AF.Square, bias=prm(P_ZERO),
                                     scale=pv(P_SQ0))
                sqB, sqBv = ct("sqB")
                nc.scalar.activation(sqBv, rho1, AF.Square, bias=prm(P_ZERO),
                                     scale=pv(P_C3))
                sqS, _ = ct("sqS")
                nc.gpsimd.tensor_add(sqS[:], sqA[:], sqB[:])
                tau, tauv = ct("tau")
                nc.gpsimd.tensor_add(tauv, tab[:, :, 0:T], tab[:, :, T:2 * T])

                # post-inv2
                isq, _ = ct("isq")
                nc.vector.tensor_mul(isq[:], inv2[:], inv2[:])
                it2, _ = ct("it2")
                nc.gpsimd.tensor_mul(it2[:], tau[:], inv2[:])
                s1, _ = ct("s1")
                nc.gpsimd.tensor_add(s1[:], n2[:], it2[:])
                qs2, _ = ct("qs2")
                nc.vector.tensor_mul(qs2[:], sqS[:], isq[:])
                n3, _ = ct("n3")
                nc.vector.tensor_add(n3[:], s1[:], qs2[:])
                inv3, inv3v = ct("inv3")
                act_raw(nc, inv3[:], n3[:], AF.Rsqrt, prm(P_EPS), 1.0 / D)

                # z / p terms (parallel with the n3/inv3 path)
                z0, z0v = ct("z0")
                nc.gpsimd.tensor_mul(z0v, rho0, inv2v)
                z1, z1v = ct("z1")
                nc.gpsimd.tensor_mul(z1v, rho1, inv2v)
                zh0, _ = ct("zh0")
                zh1, _ = ct("zh1")
                zh2, _ = ct("zh2")
                zh3, _ = ct("zh3")
                p0, p0v = ct("p0")
                p1, p1v = ct("p1")
                nc.vector.tensor_scalar(zh0[:], z0[:], pv(P_H00), None, op0=ALU.mult)
                nc.vector.tensor_scalar(zh1[:], z1[:], pv(P_H10), None, op0=ALU.mult)
                nc.vector.tensor_add(p0v, y0, zh0[:].rearrange("p (j t) -> p j t", t=T))
                nc.vector.tensor_add(p0[:], p0[:], zh1[:])
                nc.vector.tensor_scalar(zh2[:], z0[:], pv(P_H01), None, op0=ALU.mult)
                nc.vector.tensor_scalar(zh3[:], z1[:], pv(P_H11), None, op0=ALU.mult)
                nc.gpsimd.tensor_add(p1v, y1, zh2[:].rearrange("p (j t) -> p j t", t=T))
                nc.gpsimd.tensor_add(p1[:], p1[:], zh3[:])

                nc.vector.tensor_mul(lint3[:, :, 0:T], p0v, inv3v)
                nc.vector.tensor_mul(lint3[:, :, T:2 * T], p1v, inv3v)

                prev = (lint3, j0)

            # drain the last supergroup's F phase
            for q in range(NQ):
                emit_F_quad(prev[0], prev[1], q)

    nc.compile()
    return nc


_CACHE = {}


def _get_nc(PRM):
    key = PRM.tobytes()
    if _CACHE.get("key") != key:
        _CACHE["nc"] = build_bass(PRM[0])
        _CACHE["key"] = key
    return _CACHE["nc"]


def kernel(**inputs) -> np.ndarray:
    idx = np.asarray(inputs["idx"]).astype(np.int64)
    kw = {k: np.asarray(v, np.float64) for k, v in inputs.items() if k != "idx"}
    R, RHS2, SROW, PRM, CU, SW = host_tables(**kw)
    nc = _get_nc(PRM)
    in_maps = []
    for c in range(NCORES):
        bas = build_basis(idx[c * BC:(c + 1) * BC], CU, SW)
        in_maps.append({"basis": bas, "R": R, "RHS2": RHS2,
                        "SROW": SROW, "PRM": PRM})
    res = run_bass_kernel_spmd(nc, in_maps, core_ids=list(range(NCORES)))
    out = np.concatenate([np.asarray(res.results[c]["out"]).astype(np.float32)
                          for c in range(NCORES)], axis=0)
    return np.ascontiguousarray(out.reshape(B, T, V))
